# revision 3
# baseline (speedup 1.0000x reference)
"""Sliding-window (causal band) multi-head attention on 8 Trainium2 cores, v2.

Problem (hardcoded): B=2, N=2048, dim=1024, H=16, Dh=64, window=256.
  qkv = x @ W_qkv; rotary(q, k); scores = q k^T / 8 with causal band mask
  (q-256 <= k <= q); out = softmax(scores) @ v @ W_out.

Sharding: sequence-parallel. 8 cores = (batch b in 2) x (quarter qr in 4);
each core owns 512 tokens of one batch and receives a 768-token frame
(256-token halo before its chunk; zero-padded + kvalid-masked for qr=0).

v2 changes vs v1 baseline (181 us graded):
  * all DRAM tensors pre-packed host-side into the exact SBUF slab layout
    (contiguous >=1.5KB runs per partition) -> cheap HWDGE descriptor gen,
    fast startup; prefetch-all-first DMA order on the sync queue.
  * rotary reads proj results straight from PSUM (no psum->sbuf copy):
    a = pq*cos, b = pq*sin_pre with a PRE-shuffled signed sin table, so the
    rotate_half partition swap (SBUF-SBUF DMA) happens on b AFTER the
    multiply and PSUM is freed by the DVE immediately.
  * softmax denominator: 1/den via DVE reciprocal_approx_fast on a [33,512]
    tile batching both heads of a pair (rows 0/32) -> replaces 16 x 4us
    single-partition reciprocals.
  * band-mask multiplies moved to the Pool engine (DVE was saturated).
  * interleave proj group g+1 between proj g and attention g so the PE
    never waits on the rotary/exp/mask chain of the current group.
  * fp16 output store (halves store traffic); fp16 broadcast of 1/den.
"""

import numpy as np

HEADS = 16
DH = 64
WIN = 256
B = 2
N = 2048
D = 1024
CHUNK = 512          # tokens owned per core
F = CHUNK + WIN      # 768-token frame (halo + own)
NCORES = 8

# q-window (local q coords 0..512) covered by each of the 6 k-subtiles
SWIN = [(0, 128), (0, 256), (0, 384), (128, 512), (256, 512), (384, 512)]
# combined band-mask index per k-subtile (into the [128, 5, 384] mask input)
MIDX = [0, 1, 2, 2, 3, 3]

_cache = {}


def _build_program(loop_r=0):
    import concourse.bacc as bacc
    import concourse.mybir as mybir
    import concourse.tile as tile
    import concourse.bass as bass
    import contextlib

    f32 = mybir.dt.float32
    f16 = mybir.dt.float16
    Exp = mybir.ActivationFunctionType.Exp

    nc = bacc.Bacc("TRN2", target_bir_lowering=False, debug=False,
                   num_devices=NCORES)

    xT_d = nc.dram_tensor("xT", [128, 8, F], f16, kind="ExternalInput").ap()
    wqkv_d = nc.dram_tensor("W_qkv", [6, 128, 8, 512], f16,
                            kind="ExternalInput").ap()
    wout_d = nc.dram_tensor("W_out", [2, 128, 8, 512], f16,
                            kind="ExternalInput").ap()
    cos_d = nc.dram_tensor("cosT", [128, F], f16, kind="ExternalInput").ap()
    sin_d = nc.dram_tensor("sinS", [128, F], f16, kind="ExternalInput").ap()
    mc_d = nc.dram_tensor("maskc", [128, 3, 512], f16,
                          kind="ExternalInput").ap()
    kv_d = nc.dram_tensor("kvalid", [128, 6], f32, kind="ExternalInput").ap()
    yT_d = nc.dram_tensor("yT", [128, 8, CHUNK], f16, kind="ExternalOutput").ap()
    import os
    dbg = os.environ.get("DBG", "")
    if dbg:
        qdbg_d = nc.dram_tensor("qdbg", [128, 8, CHUNK], f16,
                                kind="ExternalOutput").ap()
        kdbg_d = nc.dram_tensor("kdbg", [128, 8, F], f16,
                                kind="ExternalOutput").ap()
        vdbg_d = nc.dram_tensor("vdbg", [128, HEADS, 6, DH + 1], f16,
                                kind="ExternalOutput").ap()
        odbg_d = nc.dram_tensor("odbg", [128, 8, CHUNK], f16,
                                kind="ExternalOutput").ap()
        ddbg_d = nc.dram_tensor("ddbg", [33, 8, CHUNK], f32,
                                kind="ExternalOutput").ap()

    def bcast_mid(ap2d, n):
        # [P, w] -> [P, n, w] with a stride-0 middle dim
        return bass.AP(tensor=ap2d.tensor, offset=ap2d.offset,
                       ap=[list(ap2d.ap[0]), [0, n], list(ap2d.ap[1])])

    with tile.TileContext(nc) as tc:
        _rep = contextlib.ExitStack()
        if loop_r:
            _rep.enter_context(tc.For_i(0, loop_r))
        with (
            tc.tile_pool(name="pers", bufs=1) as pers,
            tc.tile_pool(name="rot", bufs=2) as rotp,
            tc.tile_pool(name="expp", bufs=18) as expp,
            tc.tile_pool(name="norm", bufs=2) as normp,
            tc.tile_pool(name="psum_s", bufs=2, space="PSUM") as psumS,
            tc.tile_pool(name="psum_o", bufs=2, space="PSUM") as psumO,
        ):
            # ---------------- persistent SBUF ----------------
            q_sb = pers.tile([128, 8, CHUNK], f16)
            k_sb = pers.tile([128, 8, F], f16)
            v_all = pers.tile([128, HEADS, 6, DH + 1], f16)
            oh_sb = pers.tile([128, 8, CHUNK], f16)
            y_all = pers.tile([128, 8, CHUNK], f16)
            cos2 = pers.tile([128, F], f16)
            sins = pers.tile([128, F], f16)
            maskc = pers.tile([128, 3, 512], f16)
            kval = pers.tile([128, 6], f32)
            xT = pers.tile([128, 8, F], f16)
            wsl = [pers.tile([128, 8, 512], f16, name=f"w{s}") for s in range(6)]
            d33s = [pers.tile([33, CHUNK], f32, name=f"d33s{i}")
                    for i in range(2)]
            wo = [pers.tile([128, 8, 512], f16, name=f"wo{s}") for s in range(2)]

            # constants ride the scalar HWDGE ring (Pool stays
            # broadcast-only: MODIFY_POOL_CONFIG lib swaps cost ~7us)
            nc.scalar.dma_start(out=cos2, in_=cos_d)
            nc.scalar.dma_start(out=sins, in_=sin_d)
            nc.scalar.dma_start(out=kval, in_=kv_d)
            nc.scalar.dma_start(out=maskc, in_=mc_d)

            # prefetch x + all weights on sync HWDGE, first-needed first
            nc.sync.dma_start(out=xT[:, 0:1, :], in_=xT_d[:, 0:1, :])
            nc.sync.dma_start(out=wsl[0][:, 0:2, :], in_=wqkv_d[0][:, 0:2, :])
            nc.sync.dma_start(out=xT[:, 1:2, :], in_=xT_d[:, 1:2, :])
            nc.sync.dma_start(out=wsl[0][:, 2:4, :], in_=wqkv_d[0][:, 2:4, :])
            for d0 in range(2, 4):
                nc.sync.dma_start(out=xT[:, d0:d0 + 1, :],
                                  in_=xT_d[:, d0:d0 + 1, :])
            nc.sync.dma_start(out=wsl[0][:, 4:8, :], in_=wqkv_d[0][:, 4:8, :])
            for d0 in range(4, 8):
                nc.sync.dma_start(out=xT[:, d0:d0 + 1, :],
                                  in_=xT_d[:, d0:d0 + 1, :])
            for s in (2, 4, 1, 3, 5):
                nc.sync.dma_start(out=wsl[s][:, 0:4, :], in_=wqkv_d[s][:, 0:4, :])
                nc.sync.dma_start(out=wsl[s][:, 4:8, :], in_=wqkv_d[s][:, 4:8, :])
            for s in range(2):
                nc.sync.dma_start(out=wo[s], in_=wout_d[s])

            nc.vector.memset(d33s[0][:], 1.0)
            nc.vector.memset(d33s[1][:], 1.0)
            # kvalid column of v_all (65th row: denominator counts valid k)
            for t in range(6):
                nc.vector.tensor_copy(
                    v_all[:, :, t, DH:DH + 1],
                    kval[:, t:t + 1].to_broadcast([128, HEADS, 1]))

            def proj_group(g, psumP):
                pair = g // 2
                c0 = 256 * (g % 2)
                # ---- Q: 2 coltiles -> rotary a/b parts (from PSUM) ----
                wq = wsl[0 + pair]
                bq = rotp.tile([128, 2, CHUNK], f16, tag="bq", bufs=1,
                               name=f"bq{g}")
                for ch in range(2):
                    pq = psumP.tile([128, CHUNK], f32, tag="proj",
                                    name=f"pq{g}_{ch}")
                    wqc = wq[:, :, c0 + 128 * ch:c0 + 128 * (ch + 1)]
                    for d in range(8):
                        nc.tensor.matmul(pq[:], wqc[:, d, :], xT[:, d, WIN:F],
                                         start=(d == 0), stop=(d == 7))
                    nc.vector.tensor_mul(q_sb[:, 2 * g + ch, :], pq[:],
                                         cos2[:, WIN:F])
                    nc.vector.tensor_mul(bq[:, ch, :], pq[:], sins[:, WIN:F])
                # ---- K: 2 coltiles x 2 384-windows ----
                wk = wsl[2 + pair]
                bk = rotp.tile([128, 2, F], f16, tag="bk", bufs=1,
                               name=f"bk{g}")
                for win in range(2):
                    sl = slice(384 * win, 384 * (win + 1))
                    for ch in range(2):
                        pk = psumP.tile([128, 384], f32, tag="proj",
                                        name=f"pk{g}_{win}_{ch}")
                        wkc = wk[:, :, c0 + 128 * ch:c0 + 128 * (ch + 1)]
                        for d in range(8):
                            nc.tensor.matmul(pk[:], wkc[:, d, :], xT[:, d, sl],
                                             start=(d == 0), stop=(d == 7))
                        nc.vector.tensor_mul(k_sb[:, 2 * g + ch, sl], pk[:],
                                             cos2[:, sl])
                        nc.vector.tensor_mul(bk[:, ch, sl], pk[:], sins[:, sl])
                # ---- V: 6 token tiles (x stationary -> token-major v) ----
                wv = wsl[4 + pair]
                wvc = wv[:, :, c0:c0 + 256]
                for t in range(6):
                    pv = psumP.tile([128, 256], f32, tag="proj",
                                    name=f"pv{g}_{t}")
                    for d in range(8):
                        nc.tensor.matmul(pv[:], xT[:, d, 128 * t:128 * (t + 1)],
                                         wvc[:, d, :], start=(d == 0),
                                         stop=(d == 7))
                    nc.scalar.copy(
                        v_all[:, 4 * g:4 * (g + 1), t, 0:DH],
                        pv[:].rearrange("p (h e) -> p h e", h=4))
                # rotate_half partition swap via sync DMA + DVE in-place
                # adds (gpsimd compute-DMA thrashes the Pool ucode lib)
                aq = rotp.tile([128, 2, CHUNK], f16, tag="aq", bufs=1,
                               name=f"aq{g}")
                ak = rotp.tile([128, 2, F], f16, tag="ak", bufs=1,
                               name=f"ak{g}")
                for blk in range(4):
                    sp = blk ^ 1
                    nc.sync.dma_start(
                        out=aq[32 * blk:32 * (blk + 1), :, :],
                        in_=bq[32 * sp:32 * (sp + 1), :, :])
                for blk in range(4):
                    sp = blk ^ 1
                    nc.sync.dma_start(
                        out=ak[32 * blk:32 * (blk + 1), :, :],
                        in_=bk[32 * sp:32 * (sp + 1), :, :])
                nc.vector.tensor_add(q_sb[:, 2 * g:2 * (g + 1), :],
                                     q_sb[:, 2 * g:2 * (g + 1), :], aq)
                nc.vector.tensor_add(k_sb[:, 2 * g:2 * (g + 1), :],
                                     k_sb[:, 2 * g:2 * (g + 1), :], ak)

            EXI = {0: ("05", 0), 5: ("05", 128), 1: ("14", 0),
                   4: ("14", 256), 2: ("2", 0), 3: ("3", 0)}
            MKEY = {"05": 0, "14": 1, "2": 2, "3": 2}
            ext = {}

            def attn_scores(g):
                # scores + exp + mask for both head-pairs of group g
                for hp in (2 * g, 2 * g + 1):
                    for key, ii in (("05", (0, 5)), ("14", (1, 4)),
                                    ("2", (2,)), ("3", (3,))):
                        ps = psumS.tile([128, 2, 512], f32, tag="ps_s",
                                        name=f"ps{hp}_{key}")
                        for i in ii:
                            w0, w1 = SWIN[i]
                            wd = w1 - w0
                            base = EXI[i][1]
                            for hs in range(2):
                                pb = 64 * hs
                                nc.tensor.matmul(
                                    ps[:, hs, base:base + wd],
                                    k_sb[pb:pb + 64, hp, 128 * i:128 * (i + 1)],
                                    q_sb[pb:pb + 64, hp, w0:w1],
                                    start=True, stop=True)
                        mi = MKEY[key]
                        ew = sum(SWIN[i][1] - SWIN[i][0] for i in ii)
                        ex = expp.tile([128, 2, 512], f16, tag="ex",
                                       name=f"ex{hp}_{key}")
                        nc.scalar.activation(ex[:, :, :ew], ps[:, :, :ew], Exp,
                                             scale=0.125)
                        nc.vector.tensor_mul(
                            ex[:, :, :ew], ex[:, :, :ew],
                            bcast_mid(maskc[:, mi, :ew], 2))
                        ext[(hp, key)] = ex

            def attn_av(g, last=False):
                # attn@v + normalization for both head-pairs of group g
                for hp in (2 * g, 2 * g + 1):
                    fast = last and hp == 2 * g + 1
                    d33 = d33s[hp % 2]
                    pos = {}
                    for hs in range(2):
                        head = 2 * hp + hs
                        po = psumO.tile([65, CHUNK], f32, tag="ps_o",
                                        name=f"po{hp}_{hs}")
                        for j in range(4):
                            for n, i in enumerate((j, j + 1, j + 2)):
                                key, base = EXI[i]
                                off = base + 128 * j - SWIN[i][0]
                                nc.tensor.matmul(
                                    po[:, 128 * j:128 * (j + 1)],
                                    v_all[:, head, i, :],
                                    ext[(hp, key)][:, hs, off:off + 128],
                                    start=(n == 0), stop=(n == 2))
                        if not fast:
                            nc.scalar.copy(d33[32 * hs:32 * hs + 1, :],
                                           po[64:65, :])
                        pos[hs] = po
                        if fast:
                            # last head-pair: per-hs norm chain to shorten
                            # the serial tail before the output projection
                            df = normp.tile([1, CHUNK], f32, tag="df",
                                            bufs=2, name=f"df{hp}_{hs}")
                            nc.scalar.copy(df[:], po[64:65, :])
                            rf = normp.tile([1, CHUNK], f32, tag="rf",
                                            bufs=2, name=f"rf{hp}_{hs}")
                            rfh = normp.tile([1, CHUNK], f16, tag="rfh",
                                            bufs=2, name=f"rfh{hp}_{hs}")
                            nc.vector.reciprocal_approx_fast(
                                out=rf[:], in_=df[:])
                            nc.vector.tensor_copy(rfh[:], rf[:])
                            bc = normp.tile([64, CHUNK], f16, tag="bc",
                                            bufs=4, name=f"bc{hp}_{hs}")
                            nc.gpsimd.partition_broadcast(bc[:], rfh[:])
                            nc.vector.tensor_mul(
                                oh_sb[64 * hs:64 * (hs + 1), hp, :],
                                po[0:64, :], bc[:])
                    if fast:
                        continue
                    if dbg:
                        nc.sync.dma_start(out=ddbg_d[:, hp, :], in_=d33[:])
                    r33 = normp.tile([33, CHUNK], f32, tag="r33",
                                     name=f"r33_{hp}")
                    r33h = normp.tile([33, CHUNK], f16, tag="r33h",
                                      name=f"r33h_{hp}")
                    nc.vector.reciprocal_approx_fast(out=r33[:], in_=d33[:])
                    nc.vector.tensor_copy(r33h[:], r33[:])
                    for hs in range(2):
                        bc = normp.tile([64, CHUNK], f16, tag="bc", bufs=4,
                                        name=f"bc{hp}_{hs}")
                        if hs == 0:
                            src = r33h[0:1, :]
                        else:
                            # HW partition_broadcast only reads partition 0
                            src = normp.tile([1, CHUNK], f16, tag="s1",
                                             name=f"s1_{hp}")
                            nc.scalar.copy(src[:], r33h[32:33, :])
                            src = src[:]
                        nc.gpsimd.partition_broadcast(bc[:], src)
                        nc.vector.tensor_mul(
                            oh_sb[64 * hs:64 * (hs + 1), hp, :],
                            pos[hs][0:64, :], bc[:])

            with tc.tile_pool(name="psum_p", bufs=2, space="PSUM") as psumP:
                proj_group(0, psumP)
                proj_group(1, psumP)
                attn_scores(0)
                proj_group(2, psumP)
                attn_av(0)
                attn_scores(1)
                proj_group(3, psumP)
                attn_av(1)
                attn_scores(2)
            attn_av(2)
            attn_scores(3)
            attn_av(3, last=True)

            if dbg:
                nc.sync.dma_start(out=qdbg_d, in_=q_sb)
                nc.sync.dma_start(out=kdbg_d, in_=k_sb)
                nc.sync.dma_start(out=vdbg_d, in_=v_all)
                nc.sync.dma_start(out=odbg_d, in_=oh_sb)
            # ================= output projection =================
            with tc.tile_pool(name="psum_y", bufs=2, space="PSUM") as psumY:
                for og in range(2):
                    for ch in range(4):
                        o = 4 * og + ch
                        py = psumY.tile([128, CHUNK], f32, tag="ps_y",
                                        name=f"py{og}_{ch}")
                        for hp in range(8):
                            nc.tensor.matmul(
                                py[:], wo[og][:, hp, 128 * ch:128 * (ch + 1)],
                                oh_sb[:, hp, :],
                                start=(hp == 0), stop=(hp == 7))
                        nc.scalar.copy(y_all[:, o, :], py[:])
                    nc.sync.dma_start(
                        out=yT_d[:, 4 * og:4 * (og + 1), :],
                        in_=y_all[:, 4 * og:4 * (og + 1), :])

        _rep.close()
    nc.compile()
    return nc


def shard_inputs(x, rotary_emb, W_qkv, W_out):
    x = np.asarray(x, dtype=np.float32)
    rotary_emb = np.asarray(rotary_emb, dtype=np.float32)
    W_qkv = np.ascontiguousarray(np.asarray(W_qkv, dtype=np.float32))
    W_out = np.ascontiguousarray(np.asarray(W_out, dtype=np.float32))

    cos = np.cos(rotary_emb)                     # [N, 64]
    sin = np.sin(rotary_emb).copy()
    sin[:, :32] *= -1.0                          # sign-folded for rotate_half
    # padded [WIN + N, *] frames so every core slices uniformly
    xp = np.concatenate([np.zeros((B, WIN, D), np.float32), x], axis=1)
    cosp = np.concatenate([np.zeros((WIN, DH), np.float32), cos], axis=0)
    sinp = np.concatenate([np.zeros((WIN, DH), np.float32), sin], axis=0)

    # cos duplicated over the two 64-row head halves; sin pre-shuffled so
    # b = plain*sinS can be partition-swapped AFTER the multiply
    cosT = np.tile(cosp.T, (2, 1)).astype(np.float16)          # [128, F]
    perm = (np.arange(128) & 63) ^ 32
    sinS = np.tile(sinp.T, (2, 1))[perm, :].astype(np.float16)  # [128, F]

    def pack_slab(w2d):  # [1024, 512] -> [128, 8, 512]
        return np.ascontiguousarray(
            w2d.reshape(8, 128, 512).transpose(1, 0, 2)).astype(np.float16)

    W16 = W_qkv
    slabs = []
    for kind in range(3):            # q, k, v
        for pr in range(2):          # column pair
            c0 = kind * D + 512 * pr
            slabs.append(pack_slab(W16[:, c0:c0 + 512]))
    wqkv_p = np.stack(slabs)                                   # [6,128,8,512]
    wout_p = np.stack([pack_slab(W_out[:, 512 * og:512 * (og + 1)])
                       for og in range(2)])                    # [2,128,8,512]

    lo_m = np.tril(np.ones((128, 128), np.float32))   # keep r >= c
    hi_m = np.triu(np.ones((128, 128), np.float32))   # keep r <= c
    one = np.ones((128, 128), np.float32)
    maskc = np.stack([
        np.concatenate([lo_m, hi_m, one, one], axis=1),    # '05': [lo|hi|-|-]
        np.concatenate([one, lo_m, hi_m, one], axis=1),    # '14': [1|lo|hi|1]
        np.concatenate([hi_m, one, lo_m, one], axis=1),    # i2,i3: [hi|1|lo|-]
    ])
    maskc = np.ascontiguousarray(
        maskc.transpose(1, 0, 2)).astype(np.float16)  # [128, 3, 512]

    in_maps = []
    for c in range(NCORES):
        b, qr = divmod(c, 4)
        lo = CHUNK * qr                         # frame start in padded coords
        kvalid = np.ones((F,), np.float32)
        if qr == 0:
            kvalid[:WIN] = 0.0
        frame = xp[b, lo:lo + F, :]             # [768, 1024]
        xTc = np.ascontiguousarray(
            frame.T.reshape(8, 128, F).transpose(1, 0, 2)).astype(np.float16)
        in_maps.append({
            "xT": xTc,
            "cosT": np.ascontiguousarray(cosT[:, lo:lo + F]),
            "sinS": np.ascontiguousarray(sinS[:, lo:lo + F]),
            "W_qkv": wqkv_p,
            "W_out": wout_p,
            "kvalid": np.ascontiguousarray(kvalid.reshape(6, 128).T),
            "maskc": maskc,
        })
    return in_maps


def unshard(results):
    out = np.empty((B, N, D), dtype=np.float32)
    for c, r in enumerate(results):
        b, qr = divmod(c, 4)
        y = np.asarray(r["yT"], dtype=np.float32)      # [128, 8, 512]
        out[b, CHUNK * qr:CHUNK * (qr + 1), :] = (
            y.transpose(2, 1, 0).reshape(CHUNK, D))
    return out


def kernel(x, rotary_emb, W_qkv, W_out):
    from concourse.bass_utils import run_bass_kernel_spmd

    if "nc" not in _cache:
        _cache["nc"] = _build_program()
    nc = _cache["nc"]
    in_maps = shard_inputs(x, rotary_emb, W_qkv, W_out)
    res = run_bass_kernel_spmd(nc, in_maps, core_ids=list(range(NCORES)),
                               trace=False)
    return unshard(res.results)
